# revision 10
# baseline (speedup 1.0000x reference)
"""ANOVA-kernel (order 3) Trainium2 Bass kernel.

Reference computes, per batch b: sum_d e3(x[b, :, d]) where e3 is the 3rd
elementary symmetric polynomial over the F=64 fields. Newton's identities:

    e3 = (p1^3 - 3 p1 p2 + 2 p3) / 6,   p_k[b, d] = sum_f x[b, f, d]^k

so the sequential DP scan becomes three power-sum reductions:
  - p1, p2 per (b, d): DVE grouped tensor_reduce over f (x and x^2)
  - sum_d p3 per b: x^3 = x2 * x with a free per-partition accumulate
    (GPSIMD scalar_tensor_tensor overlapping DVE, or DVE
    tensor_tensor_reduce), x^2 from the Scalar engine.
  - small epilogue recombines and reduces over d via a fused accumulate.

Sharding: pure data parallel over the batch dim across 8 NeuronCores.
Each core gets 1024 batches = 8 tiles of [128 partitions x 4096 free].
"""

import numpy as np

_B, _F, _D = 8192, 64, 64
_NCORES = 8
_BP = _B // _NCORES  # batches per core
_P = 128             # partitions per tile
_FD = _F * _D        # free elems per batch

# how many tiles' p1 / p2 grouped f-reduce runs as a GPSIMD fold-tree
# (tensor_tensor adds) instead of a DVE tensor_reduce. GPSIMD overlaps the
# DVE's 1-port phases, trading ~2x element cost for parallelism.
_GPS_P1_TILES = 0
_GPS_P2_TILES = 0


def build_nc(bp=_BP, gps_p1_tiles=_GPS_P1_TILES, gps_p2_tiles=_GPS_P2_TILES):
    """Build the per-core Bass graph for bp batches.

    Inputs:  "x"   [bp, 64, 64] f32
    Outputs: "out" [128, bp/128] f32 with out[p, t] = y[t*128 + p]
    """
    from contextlib import ExitStack

    from concourse import bacc, mybir, tile

    f32 = mybir.dt.float32
    AF = mybir.ActivationFunctionType
    OP = mybir.AluOpType
    AX = mybir.AxisListType

    T = bp // _P  # tiles per core
    assert bp % _P == 0

    nc = bacc.Bacc("TRN2", target_bir_lowering=False, debug=False)
    x_ext = nc.dram_tensor("x", [bp, _F, _D], f32, kind="ExternalInput").ap()
    y_ext = nc.dram_tensor("out", [_P, T], f32, kind="ExternalOutput").ap()

    with tile.TileContext(nc) as tc, ExitStack() as ctx:
        xp = ctx.enter_context(tc.tile_pool(name="x", bufs=3))
        x2p = ctx.enter_context(tc.tile_pool(name="x2", bufs=2))
        scr = ctx.enter_context(tc.tile_pool(name="scr", bufs=1))
        pers = ctx.enter_context(tc.tile_pool(name="pers", bufs=1))

        p1b = pers.tile([_P, T * _D], f32, tag="p1b")
        p2b = pers.tile([_P, T * _D], f32, tag="p2b")
        s3 = pers.tile([_P, T], f32, tag="s3")
        eacc = pers.tile([_P, T], f32, tag="eacc")
        out8 = pers.tile([_P, T], f32, tag="out8")
        x3scr = scr.tile([_P, _FD], f32, tag="x3scr")

        def gps_fold(src, dst_slice, fb):
            """f-reduction (64 -> 1 per d) as a GPSIMD binary fold tree.

            src: [128, 4096] f-major tile; dst_slice: [128, 64] output.
            fb: [128, 2048] fold scratch.
            """
            h = _FD // 2
            nc.gpsimd.tensor_add(fb[:, :h], src[:, :h], src[:, h:])
            while h > 2 * _D:
                q = h // 2
                nc.gpsimd.tensor_add(fb[:, :q], fb[:, :q], fb[:, q:h])
                h = q
            nc.gpsimd.tensor_add(dst_slice, fb[:, :_D], fb[:, _D:2 * _D])

        xv_dram = x_ext.rearrange("(t p) f d -> t p (f d)", p=_P)
        for k in range(T):
            xt = xp.tile([_P, _FD], f32, tag="xt")
            nc.sync.dma_start(xt[:], xv_dram[k])
            x2t = x2p.tile([_P, _FD], f32, tag="x2t")
            nc.scalar.activation(x2t[:], xt[:], AF.Square)
            # p1/p2 per (b, d): grouped reduce over f (stride 64, 64 groups)
            xview = xt[:].rearrange("p (f d) -> p d f", f=_F, d=_D)
            x2view = x2t[:].rearrange("p (f d) -> p d f", f=_F, d=_D)
            if k < gps_p1_tiles:
                fb = scr.tile([_P, _FD // 2], f32, tag="fb1")
                gps_fold(xt[:], p1b[:, k * _D:(k + 1) * _D], fb)
            else:
                nc.vector.reduce_sum(p1b[:, k * _D:(k + 1) * _D], xview, axis=AX.X)
            if k < gps_p2_tiles:
                fb2 = scr.tile([_P, _FD // 2], f32, tag="fb2")
                gps_fold(x2t[:], p2b[:, k * _D:(k + 1) * _D], fb2)
            else:
                nc.vector.reduce_sum(p2b[:, k * _D:(k + 1) * _D], x2view, axis=AX.X)
            # sum_{f,d} x^3 per partition (batch): x3 = x2 * x fused w/ reduce
            # (scalar_tensor_tensor: out = (x2 * 1) * x, accum_out = sum(out))
            nc.vector.scalar_tensor_tensor(
                out=x3scr[:],
                in0=x2t[:],
                scalar=1.0,
                in1=xt[:],
                op0=OP.mult,
                op1=OP.mult,
                accum_out=s3[:, k:k + 1],
            )

        # ---- epilogue ----
        # e_term[b] = (1/6) sum_d p1 (p1^2 - 3 p2);  out = e_term + s3/3
        n = T * _D
        r = pers.tile([_P, n], f32, tag="r")
        z = pers.tile([_P, n], f32, tag="z")
        gsc = pers.tile([_P, _D], f32, tag="gsc")

        nc.vector.scalar_tensor_tensor(r[:], p1b[:], 1.0, p1b[:], OP.mult, OP.mult)
        nc.vector.scalar_tensor_tensor(z[:], p2b[:], 3.0, r[:], OP.mult, OP.subtract)
        # z = 3 p2 - p1^2 ; eacc[:, k] = sum_d (-1/6) p1 z
        for k in range(T):
            nc.vector.scalar_tensor_tensor(
                gsc[:],
                p1b[:, k * _D:(k + 1) * _D],
                -1.0 / 6.0,
                z[:, k * _D:(k + 1) * _D],
                OP.mult,
                OP.mult,
                accum_out=eacc[:, k:k + 1],
            )
        # out = eacc + s3/3
        nc.vector.scalar_tensor_tensor(
            out8[:], s3[:], 1.0 / 3.0, eacc[:], OP.mult, OP.add
        )
        nc.sync.dma_start(y_ext[:], out8[:])

    nc.compile()
    return nc


_nc_cache = {}


def _get_nc():
    key = (_BP, _GPS_P1_TILES, _GPS_P2_TILES)
    if key not in _nc_cache:
        _nc_cache[key] = build_nc(_BP, _GPS_P1_TILES, _GPS_P2_TILES)
    return _nc_cache[key]


def kernel(x: np.ndarray) -> np.ndarray:
    from concourse.bass_utils import run_bass_kernel_spmd

    x = np.ascontiguousarray(np.asarray(x, dtype=np.float32))
    assert x.shape == (_B, _F, _D), x.shape

    nc = _get_nc()
    shards = x.reshape(_NCORES, _BP, _F, _D)
    in_maps = [{"x": shards[c]} for c in range(_NCORES)]
    res = run_bass_kernel_spmd(nc, in_maps, core_ids=list(range(_NCORES)))
    outs = []
    for c in range(_NCORES):
        o = res.results[c]["out"]  # [128, T]; o[p, t] = y[t*128 + p]
        outs.append(np.asarray(o).T.reshape(-1))
    return np.concatenate(outs).reshape(_B, 1).astype(np.float32)


# revision 13
# speedup vs baseline: 1.1197x; 1.1197x over previous
"""ANOVA-kernel (order 3) Trainium2 Bass kernel.

Reference computes, per batch b: sum_d e3(x[b, :, d]) where e3 is the 3rd
elementary symmetric polynomial over the F=64 fields. Newton's identities:

    e3 = (p1^3 - 3 p1 p2 + 2 p3) / 6,   p_k[b, d] = sum_f x[b, f, d]^k

so the sequential DP scan becomes three power-sum reductions:
  - p1, p2 per (b, d): DVE grouped tensor_reduce over f (x and x^2)
  - sum_d p3 per b: x^3 = x2 * x with a free per-partition accumulate
    (GPSIMD scalar_tensor_tensor overlapping DVE, or DVE
    tensor_tensor_reduce), x^2 from the Scalar engine.
  - small epilogue recombines and reduces over d via a fused accumulate.

Sharding: pure data parallel over the batch dim across 8 NeuronCores.
Each core gets 1024 batches = 8 tiles of [128 partitions x 4096 free].
"""

import numpy as np

_B, _F, _D = 8192, 64, 64
_NCORES = 8
_BP = _B // _NCORES  # batches per core
_P = 128             # partitions per tile
_FD = _F * _D        # free elems per batch

# how many tiles' p1 / p2 grouped f-reduce runs as a GPSIMD fold-tree
# (tensor_tensor adds) instead of a DVE tensor_reduce. GPSIMD overlaps the
# DVE's 1-port phases, trading ~2x element cost for parallelism.
_GPS_P1_TILES = 0
_GPS_P2_TILES = 0


def build_nc(bp=_BP, gps_p1_tiles=_GPS_P1_TILES, gps_p2_tiles=_GPS_P2_TILES):
    """Build the per-core Bass graph for bp batches.

    Inputs:  "x"   [bp, 64, 64] f32
    Outputs: "out" [128, bp/128] f32 with out[p, t] = y[t*128 + p]
    """
    from contextlib import ExitStack

    from concourse import bacc, mybir, tile

    f32 = mybir.dt.float32
    AF = mybir.ActivationFunctionType
    OP = mybir.AluOpType
    AX = mybir.AxisListType

    T = bp // _P  # tiles per core
    assert bp % _P == 0

    # NOTE: the host passes x pre-transposed to [bp, D, F] (f innermost) so
    # the grouped f-reduces stream SBUF at unit stride (measured 1.72
    # cyc/elem at stride 256B vs ~1.0 unit-stride).
    nc = bacc.Bacc("TRN2", target_bir_lowering=False, debug=False)
    x_ext = nc.dram_tensor("x", [bp, _D, _F], f32, kind="ExternalInput").ap()
    y_ext = nc.dram_tensor("out", [_P, T], f32, kind="ExternalOutput").ap()

    with tile.TileContext(nc) as tc, ExitStack() as ctx:
        xp = ctx.enter_context(tc.tile_pool(name="x", bufs=3))
        x2p = ctx.enter_context(tc.tile_pool(name="x2", bufs=2))
        scr = ctx.enter_context(tc.tile_pool(name="scr", bufs=1))
        pers = ctx.enter_context(tc.tile_pool(name="pers", bufs=1))

        p1b = pers.tile([_P, T * _D], f32, tag="p1b")
        p2b = pers.tile([_P, T * _D], f32, tag="p2b")
        s3 = pers.tile([_P, T], f32, tag="s3")
        eacc = pers.tile([_P, T], f32, tag="eacc")
        out8 = pers.tile([_P, T], f32, tag="out8")
        x3scr = scr.tile([_P, _FD], f32, tag="x3scr")

        def gps_fold(src, dst_slice, fb):
            """f-reduction (64 -> 1 per d) as a GPSIMD binary fold tree.

            src: [128, 4096] tile in (d, f) layout; dst_slice: [128, 64].
            fb: [128, 2048] fold scratch in (d, f/2) layout. Each level adds
            the upper f-half onto the lower via 2D APs (unit-stride runs).
            """
            h = _F // 2
            sv = src.rearrange("p (d f) -> p d f", d=_D, f=_F)
            fv = fb.rearrange("p (d f) -> p d f", d=_D, f=h)
            nc.gpsimd.tensor_add(fv[:, :, :], sv[:, :, :h], sv[:, :, h:])
            while h > 2:
                q = h // 2
                nc.gpsimd.tensor_add(fv[:, :, :q], fv[:, :, :q], fv[:, :, q:h])
                h = q
            nc.gpsimd.tensor_add(dst_slice, fv[:, :, 0], fv[:, :, 1])

        xv_dram = x_ext.rearrange("(t p) d f -> t p (d f)", p=_P)
        for k in range(T):
            xt = xp.tile([_P, _FD], f32, tag="xt")
            nc.sync.dma_start(xt[:], xv_dram[k])
            x2t = x2p.tile([_P, _FD], f32, tag="x2t")
            nc.scalar.activation(x2t[:], xt[:], AF.Square)
            # p1/p2 per (b, d): grouped reduce over f (unit stride, 64 groups)
            xview = xt[:].rearrange("p (d f) -> p d f", d=_D, f=_F)
            x2view = x2t[:].rearrange("p (d f) -> p d f", d=_D, f=_F)
            if k < gps_p1_tiles:
                fb = scr.tile([_P, _FD // 2], f32, tag="fb1")
                gps_fold(xt[:], p1b[:, k * _D:(k + 1) * _D], fb[:])
            else:
                nc.vector.reduce_sum(p1b[:, k * _D:(k + 1) * _D], xview, axis=AX.X)
            if k < gps_p2_tiles:
                fb2 = scr.tile([_P, _FD // 2], f32, tag="fb2")
                gps_fold(x2t[:], p2b[:, k * _D:(k + 1) * _D], fb2[:])
            else:
                nc.vector.reduce_sum(p2b[:, k * _D:(k + 1) * _D], x2view, axis=AX.X)
            # sum_{f,d} x^3 per partition (batch): x3 = x2 * x fused w/ reduce
            # (scalar_tensor_tensor: out = (x2 * 1) * x, accum_out = sum(out))
            nc.vector.scalar_tensor_tensor(
                out=x3scr[:],
                in0=x2t[:],
                scalar=1.0,
                in1=xt[:],
                op0=OP.mult,
                op1=OP.mult,
                accum_out=s3[:, k:k + 1],
            )

        # ---- epilogue ----
        # e_term[b] = (1/6) sum_d p1 (p1^2 - 3 p2);  out = e_term + s3/3
        n = T * _D
        r = pers.tile([_P, n], f32, tag="r")
        z = pers.tile([_P, n], f32, tag="z")
        gsc = pers.tile([_P, _D], f32, tag="gsc")

        nc.vector.scalar_tensor_tensor(r[:], p1b[:], 1.0, p1b[:], OP.mult, OP.mult)
        nc.vector.scalar_tensor_tensor(z[:], p2b[:], 3.0, r[:], OP.mult, OP.subtract)
        # z = 3 p2 - p1^2 ; eacc[:, k] = sum_d (-1/6) p1 z
        for k in range(T):
            nc.vector.scalar_tensor_tensor(
                gsc[:],
                p1b[:, k * _D:(k + 1) * _D],
                -1.0 / 6.0,
                z[:, k * _D:(k + 1) * _D],
                OP.mult,
                OP.mult,
                accum_out=eacc[:, k:k + 1],
            )
        # out = eacc + s3/3
        nc.vector.scalar_tensor_tensor(
            out8[:], s3[:], 1.0 / 3.0, eacc[:], OP.mult, OP.add
        )
        nc.sync.dma_start(y_ext[:], out8[:])

    nc.compile()
    return nc


_nc_cache = {}


def _get_nc():
    key = (_BP, _GPS_P1_TILES, _GPS_P2_TILES)
    if key not in _nc_cache:
        _nc_cache[key] = build_nc(_BP, _GPS_P1_TILES, _GPS_P2_TILES)
    return _nc_cache[key]


def kernel(x: np.ndarray) -> np.ndarray:
    from concourse.bass_utils import run_bass_kernel_spmd

    x = np.ascontiguousarray(np.asarray(x, dtype=np.float32))
    assert x.shape == (_B, _F, _D), x.shape

    nc = _get_nc()
    # pre-transpose each shard to [bp, D, F] (pure layout marshaling; all
    # compute happens on-device)
    xt = np.ascontiguousarray(x.reshape(_NCORES, _BP, _F, _D).transpose(0, 1, 3, 2))
    in_maps = [{"x": xt[c]} for c in range(_NCORES)]
    res = run_bass_kernel_spmd(nc, in_maps, core_ids=list(range(_NCORES)))
    outs = []
    for c in range(_NCORES):
        o = res.results[c]["out"]  # [128, T]; o[p, t] = y[t*128 + p]
        outs.append(np.asarray(o).T.reshape(-1))
    return np.concatenate(outs).reshape(_B, 1).astype(np.float32)
